# revision 1
# baseline (speedup 1.0000x reference)
"""DLRM-ResNet forward (embedding gather + bottom/top MLP) on 8 Trainium2
NeuronCores via Bass/Tile.

Sharding: data-parallel over the batch (2048 samples/core); the 4M x 128
embedding table is replicated per core (shipped fp16 -- the table values
are uniform in [0, 2^-11] so fp16 rounding is ~1e-7 absolute, far below
fp32 matmul noise). MLP weights are replicated and run as float32r
matmuls (full-rate fp32-replicated mode, ~1e-4 relative); the embedding
contribution to the first top-MLP layer runs in fp16 (its magnitude is
~1000x below the dense-path contribution, so the rounding is invisible).

All activations live feature-major ([features, samples]) so every matmul
contracts over SBUF partitions; gathered embedding rows arrive
sample-major and are flipped with PE transposes.
"""

import os
import sys
import types
import numpy as np

import concourse.bass as bass
import concourse.tile as tile
import concourse.mybir as mybir
from concourse.bass_utils import run_bass_kernel_spmd
from concourse.masks import make_identity

f16 = mybir.dt.float16
f32 = mybir.dt.float32
f32r = mybir.dt.float32r
i32 = mybir.dt.int32

B, V, D = 16384, 4194304, 128
N_CORES = 8
BC = B // N_CORES          # samples per core
NSLOT = 26                 # sparse feature slots
BOT = [13, 256, 256, 256]
TOP = [26 * D + 256, 256, 256, 256, 256, 1]


def _split_multi_waits(nc):
    """This toolchain's walrus accepts only ONE sync-wait command per
    instruction; Tile attaches several (including on its epilogue
    Drain). Hoist the extras onto same-engine EventSemaphore
    instructions immediately before the original -- waits are
    AND-conditions, so waiting earlier on the same engine stream is
    equivalent."""
    n_new = 0
    for fn in nc.m.functions:
        for bb in fn.blocks:
            insts = list(bb.instructions)
            out = []
            changed = False
            for inst in insts:
                si = inst.sync_info
                waits = list(si.on_wait) if si is not None else []
                if len(waits) > 1:
                    changed = True
                    for w in waits[:-1]:
                        n_new += 1
                        out.append(mybir.InstEventSemaphore(
                            name=f"I-ws-{n_new}",
                            engine=inst.engine,
                            ins=[], outs=[],
                            sync_info=mybir.SyncInfo(on_wait=[w], on_update=[]),
                        ))
                    si.on_wait = [waits[-1]]
                out.append(inst)
            if changed:
                bb.instructions = out
    return n_new


def build_bass(bc=BC, v=V, tile_j=16, split_waits=True, stages=31, dma_tp=True):
    """Build the per-core Bass program. bc = samples per core (multiple of
    512), v = table rows. tile_j = bc // 128 j-subtiles."""
    assert bc % 512 == 0
    n_bt = bc // 512           # batch tiles of 512 samples
    assert tile_j == bc // 128

    nc = bass.Bass("TRN2", target_bir_lowering=False, debug=False)

    xs = nc.dram_tensor("xs", [bc, 39], f32, kind="ExternalInput")
    table = nc.dram_tensor("table", [v, D], f16, kind="ExternalInput")
    tw0e = nc.dram_tensor("tw0e", [NSLOT * D, 256], f16, kind="ExternalInput")
    tw0h = nc.dram_tensor("tw0h", [256, 256], f32, kind="ExternalInput")
    ws = {}
    for i in range(3):
        ws[f"bw{i}"] = nc.dram_tensor(f"bw{i}", [BOT[i], BOT[i + 1]], f32, kind="ExternalInput")
        ws[f"bb{i}"] = nc.dram_tensor(f"bb{i}", [BOT[i + 1], 1], f32, kind="ExternalInput")
    for i in range(5):
        if i > 0:
            ws[f"tw{i}"] = nc.dram_tensor(f"tw{i}", [TOP[i], TOP[i + 1]], f32, kind="ExternalInput")
        ws[f"tb{i}"] = nc.dram_tensor(f"tb{i}", [TOP[i + 1], 1], f32, kind="ExternalInput")
    out = nc.dram_tensor("out", [bc, 1], f32, kind="ExternalOutput")

    Relu = mybir.ActivationFunctionType.Relu
    Ident = mybir.ActivationFunctionType.Identity

    with tile.TileContext(nc) as tc:
        with tc.tile_pool(name="const", bufs=1) as cpool, \
             tc.tile_pool(name="wpool", bufs=1) as wpool, \
             tc.tile_pool(name="epool", bufs=8) as epool, \
             tc.tile_pool(name="zte", bufs=2) as ztepool, \
             tc.tile_pool(name="act", bufs=2) as apool, \
             tc.tile_pool(name="tp_ps", bufs=3, space="PSUM") as tpps, \
             tc.tile_pool(name="mm_ps", bufs=3, space="PSUM") as mmps:

            # ---- constants / weights / x ------------------------------
            ident16 = cpool.tile([128, 128], f16)
            make_identity(nc, ident16[:])
            ident32 = cpool.tile([128, 128], f32)
            make_identity(nc, ident32[:])

            x_sb = cpool.tile([128, tile_j * 39], f32)
            # sample s = 128*j + p  ->  partition p, free block j
            if stages & 64:
                nc.sync.dma_start(out=x_sb[:], in_=xs[:].rearrange("(p j) d -> p (j d)", p=128))
            else:
                nc.sync.dma_start(
                    out=x_sb[:].rearrange("p (j d) -> p j d", d=39),
                    in_=xs[:].rearrange("(j p) d -> p j d", p=128))

            # int32 indices, laid out [p, slot-major: k*tile_j + j]
            idx_f = cpool.tile([128, NSLOT * tile_j], f32)
            idx_all = cpool.tile([128, NSLOT * tile_j], i32)
            if not (stages & 128):
                nc.vector.tensor_copy(
                    out=idx_f[:].rearrange("p (k j) -> p k j", k=NSLOT),
                    in_=x_sb[:].rearrange("p (j d) -> p j d", d=39)[:, :, 13:39]
                        .rearrange("p j k -> p k j"))
                nc.vector.tensor_copy(out=idx_all[:], in_=idx_f[:])
            idx_m = cpool.tile([128, NSLOT * tile_j], i32)
            if stages & 128:
                nc.gpsimd.memset(idx_m[:], 0)
            else:
                nc.vector.tensor_scalar(
                    out=idx_m[:], in0=idx_all[:], scalar1=v - 1, scalar2=None,
                    op0=mybir.AluOpType.bitwise_and)

            load_weights = bool(stages & 8)
            tw0e_sb = wpool.tile([128, NSLOT * 256], f16)
            if load_weights:
                nc.sync.dma_start(
                    out=tw0e_sb[:].rearrange("p (k m) -> p k m", k=NSLOT),
                    in_=tw0e[:].rearrange("(k p) m -> p k m", p=128))
            tw0h_ld = wpool.tile([128, 2 * 256], f32, tag="tw0h_ld")
            if load_weights: nc.sync.dma_start(
                out=tw0h_ld[:].rearrange("p (k m) -> p k m", k=2),
                in_=tw0h[:].rearrange("(k p) m -> p k m", p=128))
            tw0h_sb = wpool.tile([128, 2 * 256], f32r)
            if load_weights: nc.vector.tensor_copy(out=tw0h_sb[:], in_=tw0h_ld[:].bitcast(f32r))
            mlp_w = {}
            for nm, fi, fo in [("bw1", 256, 256), ("bw2", 256, 256),
                               ("tw1", 256, 256), ("tw2", 256, 256),
                               ("tw3", 256, 256), ("tw4", 256, 1)]:
                tl = wpool.tile([128, (fi // 128) * fo], f32, tag=f"{nm}_ld")
                if load_weights:
                    nc.sync.dma_start(
                        out=tl[:].rearrange("p (k m) -> p k m", k=fi // 128),
                        in_=ws[nm][:].rearrange("(k p) m -> p k m", p=128))
                t = wpool.tile([128, (fi // 128) * fo], f32r, tag=nm)
                if load_weights: nc.vector.tensor_copy(out=t[:], in_=tl[:].bitcast(f32r))
                mlp_w[nm] = t[:].rearrange("p (k m) -> p k m", k=fi // 128)
            bw0_ld = wpool.tile([13, 256], f32, tag="bw0_ld")
            if load_weights: nc.sync.dma_start(out=bw0_ld[:], in_=ws["bw0"][:])
            bw0_sb = wpool.tile([13, 256], f32r)
            if load_weights: nc.vector.tensor_copy(out=bw0_sb[:], in_=bw0_ld[:].bitcast(f32r))
            biases = {}
            for nm, fo in [("bb0", 256), ("bb1", 256), ("bb2", 256),
                           ("tb0", 256), ("tb1", 256), ("tb2", 256),
                           ("tb3", 256), ("tb4", 1)]:
                t = wpool.tile([min(fo, 128), (fo + 127) // 128], f32, tag=f"b_{nm}")
                for m in range((fo + 127) // 128):
                    lo = m * 128
                    hi = min(fo, lo + 128)
                    if load_weights:
                        nc.sync.dma_start(out=t[: hi - lo, m : m + 1],
                                          in_=ws[nm][lo:hi, :])
                biases[nm] = t

            def mm_f32r(ps, w_ktiles, rhs_tiles, start, stop_at_end=True):
                """Accumulate sum_k w[k].T @ rhs[k] into ps (float32r)."""
                first = start
                n = len(w_ktiles)
                for i, (wk, rk) in enumerate(zip(w_ktiles, rhs_tiles)):
                    nc.tensor.matmul(out=ps, lhsT=wk, rhs=rk,
                                     start=first,
                                     stop=(stop_at_end and i == n - 1),
                                     skip_group_check=True)
                    first = False

            # ---- main loop over batch tiles of 512 --------------------
            for t in range(n_bt):
                # -- gather + transpose the 26 embedding slots ----------
                zte = []
                for k in range(NSLOT):
                    e_t = epool.tile([128, 512], f16, tag="ek")
                    if stages & 1:
                        for c in range(4):
                            j = 4 * t + c
                            g = k * tile_j + j
                            nc.gpsimd.indirect_dma_start(
                                out=e_t[:, c * 128:(c + 1) * 128],
                                out_offset=None,
                                in_=table[:],
                                in_offset=bass.IndirectOffsetOnAxis(
                                    ap=idx_m[:, g:g + 1], axis=0))
                    else:
                        nc.gpsimd.memset(e_t[:], 0.0)
                    zt = ztepool.tile([128, 512], f16, tag=f"zte{k}")
                    if stages & 2 and dma_tp:
                        for c in range(4):
                            nc.sync.dma_start_transpose(
                                out=zt[:, c * 128:(c + 1) * 128],
                                in_=e_t[:, c * 128:(c + 1) * 128])
                    elif stages & 2:
                        tp = tpps.tile([128, 512], f16, space="PSUM", tag="tp")
                        for c in range(4):
                            nc.tensor.transpose(
                                out=tp[:, c * 128:(c + 1) * 128],
                                in_=e_t[:, c * 128:(c + 1) * 128],
                                identity=ident16[:])
                        nc.vector.tensor_copy(out=zt[:], in_=tp[:])
                    else:
                        nc.vector.tensor_copy(out=zt[:], in_=e_t[:])
                    zte.append(zt)

                # -- dense x^T for this batch tile ----------------------
                xT = apool.tile([13, 512], f32r, tag="xT")
                if stages & 4:
                    xtp = tpps.tile([128, 512], f32, space="PSUM", tag="tp")
                    for c in range(4):
                        nc.tensor.transpose(
                            out=xtp[:39, c * 128:(c + 1) * 128],
                            in_=x_sb[:].rearrange("p (j d) -> p j d", d=39)
                                [:, 4 * t + c, :],
                            identity=ident32[:])
                    nc.vector.tensor_copy(out=xT[:], in_=xtp[:13, :].bitcast(f32r))
                else:
                    nc.gpsimd.memset(xT[:], 0.0)

                if not (stages & 8):
                    yo0 = apool.tile([1, 512], f32, tag="yo")
                    nc.vector.tensor_copy(out=yo0[:], in_=zte[0][:1, :])
                    nc.sync.dma_start(out=out[t * 512:(t + 1) * 512, :].rearrange("s o -> o s"),
                                      in_=yo0[:])
                    continue
                # -- bottom MLP (feature-major, f32r) -------------------
                h1 = apool.tile([128, 2 * 512], f32r, tag="h1")
                for m in range(2):
                    ps = mmps.tile([128, 512], f32, space="PSUM", tag="mm")
                    nc.tensor.matmul(out=ps[:], lhsT=bw0_sb[:, m * 128:(m + 1) * 128],
                                     rhs=xT[:], start=True, stop=True)
                    nc.scalar.activation(h1[:, m * 512:(m + 1) * 512], ps[:],
                                         Relu, bias=biases["bb0"][:, m:m + 1])
                hprev = h1
                for li, (wnm, bnm) in enumerate([("bw1", "bb1"), ("bw2", "bb2")]):
                    hn = apool.tile([128, 2 * 512], f32r, tag=f"h{li + 2}")
                    for m in range(2):
                        ps = mmps.tile([128, 512], f32, space="PSUM", tag="mm")
                        mm_f32r(ps[:], [mlp_w[wnm][:, kk, m * 128:(m + 1) * 128] for kk in range(2)],
                                [hprev[:, kk * 512:(kk + 1) * 512] for kk in range(2)], True)
                        rl = apool.tile([128, 512], f32r, tag="rl")
                        nc.scalar.activation(rl[:], ps[:], Relu,
                                             bias=biases[bnm][:, m:m + 1])
                        nc.vector.tensor_add(out=hn[:, m * 512:(m + 1) * 512],
                                             in0=rl[:], in1=hprev[:, m * 512:(m + 1) * 512])
                    hprev = hn

                # -- top MLP layer 0 (h part f32r + emb part f16) -------
                z0 = apool.tile([128, 2 * 512], f32r, tag="z0")
                for m in range(2):
                    ps = mmps.tile([128, 512], f32, space="PSUM", tag="mm")
                    mm_f32r(ps[:], [tw0h_sb[:].rearrange("p (k m) -> p k m", k=2)[:, kk, m * 128:(m + 1) * 128] for kk in range(2)],
                            [hprev[:, kk * 512:(kk + 1) * 512] for kk in range(2)], True,
                            stop_at_end=False)
                    for k in range(NSLOT):
                        nc.tensor.matmul(
                            out=ps[:],
                            lhsT=tw0e_sb[:].rearrange("p (k m) -> p k m", k=NSLOT)[:, k, m * 128:(m + 1) * 128],
                            rhs=zte[k][:], start=False, stop=(k == NSLOT - 1),
                            skip_group_check=True)
                    nc.scalar.activation(z0[:, m * 512:(m + 1) * 512], ps[:],
                                         Relu, bias=biases["tb0"][:, m:m + 1])
                zprev = z0
                for li, (wnm, bnm) in enumerate([("tw1", "tb1"), ("tw2", "tb2"), ("tw3", "tb3")]):
                    zn = apool.tile([128, 2 * 512], f32r, tag=f"z{li + 1}")
                    for m in range(2):
                        ps = mmps.tile([128, 512], f32, space="PSUM", tag="mm")
                        mm_f32r(ps[:], [mlp_w[wnm][:, kk, m * 128:(m + 1) * 128] for kk in range(2)],
                                [zprev[:, kk * 512:(kk + 1) * 512] for kk in range(2)], True)
                        rl = apool.tile([128, 512], f32r, tag="rl")
                        nc.scalar.activation(rl[:], ps[:], Relu,
                                             bias=biases[bnm][:, m:m + 1])
                        nc.vector.tensor_add(out=zn[:, m * 512:(m + 1) * 512],
                                             in0=rl[:], in1=zprev[:, m * 512:(m + 1) * 512])
                    zprev = zn

                # -- final layer [256 -> 1] -----------------------------
                ps = mmps.tile([128, 512], f32, space="PSUM", tag="mm")
                mm_f32r(ps[:1, :], [mlp_w["tw4"][:, kk, 0:1] for kk in range(2)],
                        [zprev[:, kk * 512:(kk + 1) * 512] for kk in range(2)], True)
                yo = apool.tile([1, 512], f32, tag="yo")
                nc.scalar.activation(yo[:], ps[:1, :], Ident,
                                     bias=biases["tb4"][:, 0:1])
                nc.sync.dma_start(out=out[t * 512:(t + 1) * 512, :].rearrange("s o -> o s"),
                                  in_=yo[:])

    if split_waits:
        _split_multi_waits(nc)
    return nc


_NC_CACHE = {}


def _get_nc(bc=BC, v=V):
    key = (bc, v)
    if key not in _NC_CACHE:
        _NC_CACHE[key] = build_bass(bc=bc, v=v, tile_j=bc // 128)
    return _NC_CACHE[key]


def make_in_maps(inputs, bc=BC, v=V, n_cores=N_CORES):
    x = np.ascontiguousarray(np.asarray(inputs["x"], dtype=np.float32))
    table = np.concatenate([np.asarray(inputs[f"emb{i}"]) for i in range(4)],
                           axis=0).astype(np.float16)
    assert table.shape[0] == v
    tw0 = np.asarray(inputs["tw0"], dtype=np.float32)
    shared = {
        "table": table,
        "tw0e": np.ascontiguousarray(tw0[256:]).astype(np.float16),
        "tw0h": np.ascontiguousarray(tw0[:256]),
    }
    for i in range(3):
        shared[f"bw{i}"] = np.asarray(inputs[f"bw{i}"], dtype=np.float32)
        shared[f"bb{i}"] = np.asarray(inputs[f"bb{i}"], dtype=np.float32).reshape(-1, 1)
    for i in range(5):
        if i > 0:
            shared[f"tw{i}"] = np.asarray(inputs[f"tw{i}"], dtype=np.float32)
        shared[f"tb{i}"] = np.asarray(inputs[f"tb{i}"], dtype=np.float32).reshape(-1, 1)
    return [dict(shared, xs=np.ascontiguousarray(x[c * bc:(c + 1) * bc]))
            for c in range(n_cores)]


LAST_EXEC_NS = None


def kernel(**inputs) -> np.ndarray:
    global LAST_EXEC_NS
    nc = _get_nc()
    in_maps = make_in_maps(inputs)
    trace = bool(int(os.environ.get("KERNEL_TRACE", "0")))
    res = run_bass_kernel_spmd(nc, in_maps, core_ids=list(range(N_CORES)),
                               trace=trace)
    LAST_EXEC_NS = res.exec_time_ns
    return np.concatenate([res.results[c]["out"] for c in range(N_CORES)], axis=0)

